# revision 2
# baseline (speedup 1.0000x reference)
# Trainium2 Bass kernel for nn_LorentzSparseSqDisAtt (GNN edge attention), v3.
#
# Algorithm (8 cores, full I/O):
#   Nodes padded to 53248 and split into 8 blocks of 6656. Edges sharded by
#   src-block (core c owns edges with src in block c), bucketed by dst-block.
#   Phase 1 (sharded, transposed): core c builds the feature-major table
#     for its own block: muT = W @ xt via PE (static weights), per-node
#     scalar chain on [1,512] rows, per-node scales broadcast to 64 rows
#     via K=1 matmuls. Table rows: 0..63 tail, 64 y0, 65 const 1.0,
#     66..79 zero. AllGather -> full table on every core.
#   Phase 2: per dst-block: load block table to SBUF, gpsimd.ap_gather both
#     endpoints (block-relative idx fit int16), DVE product, PE ones-matmul
#     with weights [-1 x64, +1, -1] gives t = y0s*y0d - ts.td - 1 directly,
#     DVE clip, ACT exp -> f16.
#   All repetitive work runs under tc.For_i hardware loops: per-program-
#   instruction dispatch costs ~60us in this runtime, so the program is
#   kept to ~100 instructions.
import numpy as np

N = 50000
DSP = 64          # spatial dim
NCORES = 8
BLK = 6656        # 13 * 512 nodes per block
NPAD = BLK * NCORES
NW = BLK // 512   # phase-1 iterations (512 nodes each)
CAP = 14336       # max edges per gather tile (SBUF bound)

_prog_cache = {}
REPS = 1


def _build_program(E_gt, T, bias_nonzero, reps=1):
    from contextlib import ExitStack

    import concourse.bacc as bacc
    import concourse.bass as bass
    import concourse.tile as tile
    from concourse import mybir
    from concourse.bass import ds, ts

    f32 = mybir.dt.float32
    f16 = mybir.dt.float16
    i16 = mybir.dt.int16
    AF = mybir.ActivationFunctionType
    OP = mybir.AluOpType

    GW = E_gt // 16
    NT = NCORES * T           # gather tiles per core
    NQ = E_gt // 512          # matmul chunks per tile

    nc = bacc.Bacc(
        "TRN2",
        target_bir_lowering=False,
        debug=False,
        enable_asserts=False,
        num_devices=NCORES,
    )

    xT = nc.dram_tensor("xT", [DSP + 1, BLK], f32, kind="ExternalInput").ap()
    wt = nc.dram_tensor("wt", [DSP, DSP], f32, kind="ExternalInput").ap()
    bias_d = nc.dram_tensor("bias", [1, DSP], f32, kind="ExternalInput").ap()
    idxs_d = nc.dram_tensor("idxs", [NT * 16, GW], i16,
                            kind="ExternalInput").ap()
    idxd_d = nc.dram_tensor("idxd", [NT * 16, GW], i16,
                            kind="ExternalInput").ap()
    wv_d = nc.dram_tensor("wv", [66, 1], f32, kind="ExternalInput").ap()
    bc2_d = nc.dram_tensor("bc2", [2, DSP + 1], f32, kind="ExternalInput").ap()
    res = nc.dram_tensor("res", [NT, E_gt], f16, kind="ExternalOutput").ap()
    tbl_in = nc.dram_tensor("tbl_in", [66, BLK], f32).ap()
    tbl_all = nc.dram_tensor(
        "tbl_all", [NCORES * 66, BLK], f32, addr_space="Shared"
    ).ap()

    with tile.TileContext(nc) as tc, ExitStack() as ctx:
        cpool = ctx.enter_context(tc.tile_pool(name="const", bufs=1))

        tblT = cpool.tile([80, BLK], f32)       # my block, feature-major
        nc.gpsimd.memset(tblT[64:80, :], 0.0)
        o_row = cpool.tile([1, BLK], f32)
        nc.gpsimd.memset(o_row[:], 1.0)
        nc.sync.dma_start(tblT[65:66, :], o_row[:])   # table row 65 == 1.0
        wvec = cpool.tile([66, 1], f32)
        nc.sync.dma_start(wvec[:], wv_d)
        neg1 = cpool.tile([1, 1], f32)
        nc.gpsimd.memset(neg1[:], -1.0)
        ones64 = cpool.tile([DSP, 1], f32)
        nc.gpsimd.memset(ones64[:], 1.0)
        onesr = cpool.tile([1, DSP], f32)
        nc.gpsimd.memset(onesr[:], 1.0)
        bc2 = cpool.tile([2, DSP + 1], f32)
        nc.sync.dma_start(bc2[:], bc2_d)
        wt_t = cpool.tile([DSP, DSP], f32)
        nc.sync.dma_start(wt_t[:], wt)

        # ---------------- Phase 1: build my block's table ----------------
        with ExitStack() as p1ctx:
            p1 = p1ctx.enter_context(tc.tile_pool(name="p1", bufs=1))
            pps = p1ctx.enter_context(
                tc.tile_pool(name="pps", bufs=1, space="PSUM")
            )
            p1c = p1ctx.enter_context(tc.tile_pool(name="p1c", bufs=1))

            xts = p1c.tile([DSP, BLK], f32)
            nc.sync.dma_start(xts[:], xT[1 : DSP + 1, :])
            x0r = p1c.tile([1, BLK], f32)
            nc.sync.dma_start(x0r[:], xT[0:1, :])
            if bias_nonzero:
                # b_col[f] = bias[f]: transpose the [1,64] row via PE
                b_row = p1c.tile([1, DSP], f32)
                nc.sync.dma_start(b_row[:], bias_d)
                one1 = p1c.tile([1, 1], f32)
                nc.gpsimd.memset(one1[:], 1.0)
                bT = pps.tile([DSP, 1], f32, tag="bT")
                nc.tensor.matmul(bT[:], lhsT=b_row[:], rhs=one1[:],
                                 start=True, stop=True)
                b_col = p1c.tile([DSP, 1], f32)
                nc.vector.tensor_copy(b_col[:], bT[:])

            mu2e = p1.tile([DSP + 1, 512], f32)
            nc.gpsimd.memset(mu2e[64:65, :], 1.0)
            sny0 = p1.tile([2, 512], f32)

            # Single For_i body; ACT uses only {Ln, Exp} (one table set):
            # sqrt(x) computed as exp(0.5*ln(x)).
            with tc.For_i(0, NW) as i:
                x0c = x0r[:, ts(i, 512)]             # [1, 512]
                z = p1.tile([1, 512], f32, tag="z")
                nc.vector.tensor_scalar_max(z[:], x0c, 1.0 + 1e-7)
                zsq = p1.tile([1, 512], f32, tag="zsq")
                nc.vector.tensor_tensor(out=zsq[:], in0=z[:], in1=z[:],
                                        op=OP.mult)
                lnu = p1.tile([1, 512], f32, tag="lnu")
                nc.scalar.activation(lnu[:], zsq[:], AF.Ln, bias=neg1[:])
                w0 = p1.tile([1, 512], f32, tag="w0")
                nc.scalar.activation(w0[:], lnu[:], AF.Exp, scale=0.5)
                zw = p1.tile([1, 512], f32, tag="zw")
                nc.vector.tensor_tensor(out=zw[:], in0=z[:], in1=w0[:],
                                        op=OP.add)
                dist = p1.tile([1, 512], f32, tag="dist")
                nc.scalar.activation(dist[:], zw[:], AF.Ln)
                wc = p1.tile([1, 512], f32, tag="wc")
                nc.vector.tensor_scalar_max(wc[:], w0[:], 1e-10)
                wci = p1.tile([1, 512], f32, tag="wci")
                nc.vector.reciprocal(wci[:], wc[:])
                snd = p1.tile([1, 512], f32, tag="snd")
                nc.vector.tensor_tensor(out=snd[:], in0=dist[:], in1=wci[:],
                                        op=OP.mult)

                muT = pps.tile([DSP, 512], f32, tag="muT")
                nc.tensor.matmul(
                    muT[:], lhsT=wt_t[:], rhs=xts[:, ts(i, 512)],
                    start=True, stop=True,
                )
                nc.vector.tensor_copy(mu2e[0:DSP, :], muT[:])
                if bias_nonzero:
                    sbc = pps.tile([DSP, 512], f32, tag="sbc")
                    nc.tensor.matmul(sbc[:], lhsT=onesr[:], rhs=snd[:],
                                     start=True, stop=True)
                    nc.vector.tensor_tensor(out=mu2e[0:DSP, :],
                                            in0=mu2e[0:DSP, :],
                                            in1=sbc[:], op=OP.mult)
                    nc.vector.tensor_scalar(
                        out=mu2e[0:DSP, :], in0=mu2e[0:DSP, :],
                        scalar1=b_col[:], scalar2=None, op0=OP.add,
                    )
                sq = p1.tile([DSP, 512], f32, tag="sq")
                nc.vector.tensor_tensor(out=sq[:], in0=mu2e[0:DSP, :],
                                        in1=mu2e[0:DSP, :], op=OP.mult)
                msq = pps.tile([1, 512], f32, tag="msq")
                nc.tensor.matmul(msq[:], lhsT=ones64[:], rhs=sq[:],
                                 start=True, stop=True)

                lnm = p1.tile([1, 512], f32, tag="lnm")
                nc.scalar.activation(lnm[:], msq[:], AF.Ln)
                r0 = p1.tile([1, 512], f32, tag="r0")
                nc.scalar.activation(r0[:], lnm[:], AF.Exp, scale=0.5)
                if not bias_nonzero:
                    nc.vector.tensor_tensor(out=r0[:], in0=r0[:],
                                            in1=snd[:], op=OP.mult)
                rc = p1.tile([1, 512], f32, tag="rc")
                nc.vector.tensor_scalar_max(rc[:], r0[:], 1e-10)
                ep = p1.tile([1, 512], f32, tag="ep")
                nc.scalar.activation(ep[:], rc[:], AF.Exp)
                em = p1.tile([1, 512], f32, tag="em")
                nc.scalar.activation(em[:], rc[:], AF.Exp, scale=-1.0)
                y0 = p1.tile([1, 512], f32, tag="y0")
                nc.vector.tensor_tensor(out=y0[:], in0=ep[:], in1=em[:],
                                        op=OP.add)
                nc.vector.tensor_scalar_mul(y0[:], y0[:], 0.5)
                f0 = p1.tile([1, 512], f32, tag="f0")
                nc.vector.tensor_tensor(out=f0[:], in0=ep[:], in1=em[:],
                                        op=OP.subtract)
                rci = p1.tile([1, 512], f32, tag="rci")
                nc.vector.reciprocal(rci[:], rc[:])
                g = p1.tile([1, 512], f32, tag="g")
                nc.vector.tensor_tensor(out=g[:], in0=f0[:], in1=rci[:],
                                        op=OP.mult)
                if not bias_nonzero:
                    nc.vector.tensor_tensor(out=g[:], in0=g[:], in1=snd[:],
                                            op=OP.mult)
                nc.vector.tensor_scalar_mul(sny0[0:1, :], g[:], 0.5)
                nc.sync.dma_start(sny0[1:2, :], y0[:])

                # bc rows 0..63 = sn broadcast, row 64 = y0
                bc = pps.tile([DSP + 1, 512], f32, tag="bc")
                nc.tensor.matmul(bc[:], lhsT=bc2[:], rhs=sny0[:],
                                 start=True, stop=True)
                nc.vector.tensor_tensor(
                    out=tblT[0 : DSP + 1, ts(i, 512)], in0=mu2e[:],
                    in1=bc[:], op=OP.mult,
                )

            nc.sync.dma_start(tbl_in, tblT[0:66, :])

        tc.strict_bb_all_engine_barrier()
        nc.gpsimd.collective_compute(
            "AllGather",
            mybir.AluOpType.bypass,
            replica_groups=[list(range(NCORES))],
            ins=[tbl_in],
            outs=[tbl_all],
        )
        tc.strict_bb_all_engine_barrier()

        # ---------------- Phase 2: gather + Lorentz inner product ----------
        pg = ctx.enter_context(tc.tile_pool(name="pg", bufs=1))
        pq = ctx.enter_context(tc.tile_pool(name="pq", bufs=1, space="PSUM"))

        A = pg.tile([80, E_gt], f32)
        B = pg.tile([80, E_gt], f32)
        Tdst = pg.tile([80, BLK], f32)
        nc.gpsimd.memset(Tdst[64:80, :], 0.0)
        ia = pg.tile([80, GW], i16)
        ib = pg.tile([80, GW], i16)
        rs = pg.tile([1, E_gt], f16)

        def p2_body(nt, tdst_src):
            nc.sync.dma_start(Tdst[0:66, :], tdst_src)
            nc.sync.dma_start(ia[0:16, :], idxs_d[ts(nt, 16), :])
            nc.sync.dma_start(ib[0:16, :], idxd_d[ts(nt, 16), :])
            for t in (ia, ib):
                nc.sync.dma_start(t[16:32, :], t[0:16, :])
                nc.sync.dma_start(t[32:64, :], t[0:32, :])
                nc.sync.dma_start(t[64:80, :], t[0:16, :])
            nc.gpsimd.ap_gather(
                B[:], Tdst[:], ib[:],
                channels=80, num_elems=BLK, d=1, num_idxs=E_gt,
            )
            nc.gpsimd.ap_gather(
                A[:], tblT[:], ia[:],
                channels=80, num_elems=BLK, d=1, num_idxs=E_gt,
            )
            nc.vector.tensor_tensor(
                out=A[0:66, :], in0=A[0:66, :], in1=B[0:66, :], op=OP.mult
            )
            NR = NQ // 8          # full rounds of 8 chunks
            def round_body(roff, rdst, nch):
                ps = pq.tile([1, 4096], f32, tag="ps")
                for k in range(nch):
                    nc.tensor.matmul(
                        ps[:, k * 512 : (k + 1) * 512], lhsT=wvec[:],
                        rhs=roff[:, k * 512 : (k + 1) * 512],
                        start=True, stop=True,
                    )
                w = nch * 512
                nc.vector.tensor_scalar(
                    out=ps[:, 0:w], in0=ps[:, 0:w], scalar1=1e-10,
                    scalar2=1.0, op0=OP.max, op1=OP.min,
                )
                nc.scalar.activation(rdst, ps[:, 0:w], AF.Exp, scale=-1.0)

            if NR:
                with tc.For_i(0, NR) as r:
                    round_body(A[0:66, ts(r, 4096)], rs[:, ts(r, 4096)], 8)
            if NQ % 8:
                round_body(A[0:66, NR * 4096 : NQ * 512],
                           rs[:, NR * 4096 : NQ * 512], NQ % 8)
            nc.sync.dma_start(res[ds(nt, 1), :], rs[:])

        if T == 1:
            for _ in range(reps):
                with tc.For_i(0, NT) as nt:
                    p2_body(nt, tbl_all[ts(nt, 66), :])
        else:
            for j in range(NCORES):
                with tc.For_i(j * T, (j + 1) * T) as nt:
                    p2_body(nt, tbl_all[j * 66 : (j + 1) * 66, :])

    nc.compile()
    return nc


def kernel(x, weight, bias, adj_indices):
    from concourse.bass_utils import run_bass_kernel_spmd

    x = np.asarray(x, dtype=np.float32)
    weight = np.asarray(weight, dtype=np.float32)
    bias_np = np.asarray(bias, dtype=np.float32)
    adj = np.asarray(adj_indices)
    Eall = adj.shape[1]
    src = adj[0].astype(np.int64)
    dst = adj[1].astype(np.int64)

    # ---- host prep: bucket edges by (src block, dst block) ----
    key = (src // BLK) * NCORES + (dst // BLK)
    order = np.argsort(key, kind="stable")
    counts = np.bincount(key, minlength=NCORES * NCORES).reshape(
        NCORES, NCORES
    )
    starts = np.zeros(NCORES * NCORES + 1, dtype=np.int64)
    np.cumsum(counts.reshape(-1), out=starts[1:])
    E_gmax = int(counts.max())
    T = max(1, -(-E_gmax // CAP))
    per_t = -(-E_gmax // T)
    E_gt = max(-(-per_t // 512) * 512, 512)
    GW = E_gt // 16
    NT = NCORES * T

    # ---- per-core inputs ----
    xp = np.zeros((NPAD, DSP + 1), dtype=np.float32)
    xp[:N] = x
    xp[N:, 0] = 1.0
    xT = np.ascontiguousarray(xp.T)                       # [65, NPAD]
    wt = np.ascontiguousarray(weight.T)                   # [k, j]
    b_in = np.ascontiguousarray(bias_np.reshape(1, DSP))
    wv_host = np.full((66, 1), -1.0, dtype=np.float32)
    wv_host[64, 0] = 1.0
    bc2_host = np.zeros((2, DSP + 1), dtype=np.float32)
    bc2_host[0, 0:DSP] = 1.0
    bc2_host[1, DSP] = 1.0
    bias_nonzero = bool(np.any(bias_np != 0.0))

    in_maps = []
    sels = []
    for c in range(NCORES):
        idx_s = np.zeros((NT, 16, GW), dtype=np.int16)
        idx_d = np.zeros((NT, 16, GW), dtype=np.int16)
        sel_c = []
        for j in range(NCORES):
            k = c * NCORES + j
            cnt = int(counts[c, j])
            sel = order[starts[k] : starts[k] + cnt]
            s_rel = (src[sel] - c * BLK).astype(np.int16)
            d_rel = (dst[sel] - j * BLK).astype(np.int16)
            for t in range(T):
                lo, hi = t * E_gt, min((t + 1) * E_gt, cnt)
                nt = j * T + t
                if lo >= cnt:
                    sel_c.append((nt, None))
                    continue
                n = hi - lo
                sp = np.zeros(E_gt, dtype=np.int16)
                dp = np.zeros(E_gt, dtype=np.int16)
                sp[:n] = s_rel[lo:hi]
                dp[:n] = d_rel[lo:hi]
                idx_s[nt] = sp.reshape(GW, 16).T
                idx_d[nt] = dp.reshape(GW, 16).T
                sel_c.append((nt, sel[lo:hi]))
        sels.append(sel_c)
        c0 = c * BLK
        in_maps.append({
            "xT": np.ascontiguousarray(xT[:, c0 : c0 + BLK]),
            "wt": wt,
            "bias": b_in,
            "idxs": idx_s.reshape(NT * 16, GW),
            "idxd": idx_d.reshape(NT * 16, GW),
            "wv": wv_host,
            "bc2": bc2_host,
        })

    key_p = (E_gt, T, bias_nonzero, REPS)
    if key_p not in _prog_cache:
        _prog_cache[key_p] = _build_program(E_gt, T, bias_nonzero, REPS)
    nc = _prog_cache[key_p]

    import sys

    _self = sys.modules[__name__]  # stash run args/results for the harness
    _self.LAST_ARGS = (nc, in_maps)
    robj = run_bass_kernel_spmd(nc, in_maps, list(range(NCORES)))
    _self.LAST_RUN = robj
    results = robj.results

    # ---- host reassembly ----
    out = np.empty(Eall, dtype=np.float32)
    for c in range(NCORES):
        r = results[c]["res"]  # [NT, E_gt] f16
        for nt, sel in sels[c]:
            if sel is None:
                continue
            out[sel] = r[nt, : len(sel)].astype(np.float32)
    return out


# revision 3
# speedup vs baseline: 1.2096x; 1.2096x over previous
# Trainium2 Bass kernel for nn_LorentzSparseSqDisAtt (GNN edge attention), v3.
#
# Algorithm (8 cores, full I/O):
#   Nodes padded to 53248 and split into 8 blocks of 6656. Edges sharded by
#   src-block (core c owns edges with src in block c), bucketed by dst-block.
#   Phase 1 (sharded, transposed): core c builds the feature-major table
#     for its own block: muT = W @ xt via PE (static weights), per-node
#     scalar chain on [1,512] rows, per-node scales broadcast to 64 rows
#     via K=1 matmuls. Table rows: 0..63 tail, 64 y0, 65 const 1.0,
#     66..79 zero. AllGather -> full table on every core.
#   Phase 2: per dst-block: load block table to SBUF, gpsimd.ap_gather both
#     endpoints (block-relative idx fit int16), DVE product, PE ones-matmul
#     with weights [-1 x64, +1, -1] gives t = y0s*y0d - ts.td - 1 directly,
#     DVE clip, ACT exp -> f16.
#   All repetitive work runs under tc.For_i hardware loops: per-program-
#   instruction dispatch costs ~60us in this runtime, so the program is
#   kept to ~100 instructions.
import numpy as np

N = 50000
DSP = 64          # spatial dim
NCORES = 8
BLK = 6656        # 13 * 512 nodes per block
NPAD = BLK * NCORES
NW = BLK // 512   # phase-1 iterations (512 nodes each)
CAP = 14336       # max edges per gather tile (SBUF bound)

_prog_cache = {}
REPS = 1


def _build_program(E_gt, T, bias_nonzero, reps=1):
    from contextlib import ExitStack

    import concourse.bacc as bacc
    import concourse.bass as bass
    import concourse.tile as tile
    from concourse import mybir
    from concourse.bass import ds, ts

    f32 = mybir.dt.float32
    f16 = mybir.dt.float16
    i16 = mybir.dt.int16
    AF = mybir.ActivationFunctionType
    OP = mybir.AluOpType

    GW = E_gt // 16
    NT = NCORES * T           # gather tiles per core
    NQ = E_gt // 512          # matmul chunks per tile

    nc = bacc.Bacc(
        "TRN2",
        target_bir_lowering=False,
        debug=False,
        enable_asserts=False,
        num_devices=NCORES,
    )

    xT = nc.dram_tensor("xT", [DSP + 1, BLK], f32, kind="ExternalInput").ap()
    wt = nc.dram_tensor("wt", [DSP, DSP], f32, kind="ExternalInput").ap()
    bias_d = nc.dram_tensor("bias", [1, DSP], f32, kind="ExternalInput").ap()
    idxs_d = nc.dram_tensor("idxs", [NT * 16, GW], i16,
                            kind="ExternalInput").ap()
    idxd_d = nc.dram_tensor("idxd", [NT * 16, GW], i16,
                            kind="ExternalInput").ap()
    wv_d = nc.dram_tensor("wv", [66, 1], f32, kind="ExternalInput").ap()
    bc2_d = nc.dram_tensor("bc2", [2, DSP + 1], f32, kind="ExternalInput").ap()
    res = nc.dram_tensor("res", [NT, E_gt], f16, kind="ExternalOutput").ap()
    tbl_in = nc.dram_tensor("tbl_in", [66, BLK], f32).ap()
    tbl_all = nc.dram_tensor(
        "tbl_all", [NCORES * 66, BLK], f32, addr_space="Shared"
    ).ap()

    with tile.TileContext(nc) as tc, ExitStack() as ctx:
        cpool = ctx.enter_context(tc.tile_pool(name="const", bufs=1))

        tblT = cpool.tile([80, BLK], f32)       # my block, feature-major
        nc.gpsimd.memset(tblT[64:80, :], 0.0)
        o_row = cpool.tile([1, BLK], f32)
        nc.gpsimd.memset(o_row[:], 1.0)
        nc.sync.dma_start(tblT[65:66, :], o_row[:])   # table row 65 == 1.0
        wvec = cpool.tile([66, 1], f32)
        nc.sync.dma_start(wvec[:], wv_d)
        neg1 = cpool.tile([1, 1], f32)
        nc.gpsimd.memset(neg1[:], -1.0)
        ones64 = cpool.tile([DSP, 1], f32)
        nc.gpsimd.memset(ones64[:], 1.0)
        onesr = cpool.tile([1, DSP], f32)
        nc.gpsimd.memset(onesr[:], 1.0)
        bc2 = cpool.tile([2, DSP + 1], f32)
        nc.sync.dma_start(bc2[:], bc2_d)
        wt_t = cpool.tile([DSP, DSP], f32)
        nc.sync.dma_start(wt_t[:], wt)

        # ---------------- Phase 1: build my block's table ----------------
        with ExitStack() as p1ctx:
            p1 = p1ctx.enter_context(tc.tile_pool(name="p1", bufs=1))
            pps = p1ctx.enter_context(
                tc.tile_pool(name="pps", bufs=1, space="PSUM")
            )
            p1c = p1ctx.enter_context(tc.tile_pool(name="p1c", bufs=1))

            xts = p1c.tile([DSP, BLK], f32)
            nc.sync.dma_start(xts[:], xT[1 : DSP + 1, :])
            x0r = p1c.tile([1, BLK], f32)
            nc.sync.dma_start(x0r[:], xT[0:1, :])
            if bias_nonzero:
                # b_col[f] = bias[f]: transpose the [1,64] row via PE
                b_row = p1c.tile([1, DSP], f32)
                nc.sync.dma_start(b_row[:], bias_d)
                one1 = p1c.tile([1, 1], f32)
                nc.gpsimd.memset(one1[:], 1.0)
                bT = pps.tile([DSP, 1], f32, tag="bT")
                nc.tensor.matmul(bT[:], lhsT=b_row[:], rhs=one1[:],
                                 start=True, stop=True)
                b_col = p1c.tile([DSP, 1], f32)
                nc.vector.tensor_copy(b_col[:], bT[:])

            mu2e = p1.tile([DSP + 1, 512], f32)
            nc.gpsimd.memset(mu2e[64:65, :], 1.0)
            sny0 = p1.tile([2, 512], f32)

            # Single For_i body; ACT uses only {Ln, Exp} (one table set):
            # sqrt(x) computed as exp(0.5*ln(x)).
            with tc.For_i(0, NW) as i:
                x0c = x0r[:, ts(i, 512)]             # [1, 512]
                z = p1.tile([1, 512], f32, tag="z")
                nc.vector.tensor_scalar_max(z[:], x0c, 1.0 + 1e-7)
                zsq = p1.tile([1, 512], f32, tag="zsq")
                nc.vector.tensor_tensor(out=zsq[:], in0=z[:], in1=z[:],
                                        op=OP.mult)
                lnu = p1.tile([1, 512], f32, tag="lnu")
                nc.scalar.activation(lnu[:], zsq[:], AF.Ln, bias=neg1[:])
                w0 = p1.tile([1, 512], f32, tag="w0")
                nc.scalar.activation(w0[:], lnu[:], AF.Exp, scale=0.5)
                zw = p1.tile([1, 512], f32, tag="zw")
                nc.vector.tensor_tensor(out=zw[:], in0=z[:], in1=w0[:],
                                        op=OP.add)
                dist = p1.tile([1, 512], f32, tag="dist")
                nc.scalar.activation(dist[:], zw[:], AF.Ln)
                wc = p1.tile([1, 512], f32, tag="wc")
                nc.vector.tensor_scalar_max(wc[:], w0[:], 1e-10)
                wci = p1.tile([1, 512], f32, tag="wci")
                nc.vector.reciprocal(wci[:], wc[:])
                snd = p1.tile([1, 512], f32, tag="snd")
                nc.vector.tensor_tensor(out=snd[:], in0=dist[:], in1=wci[:],
                                        op=OP.mult)

                muT = pps.tile([DSP, 512], f32, tag="muT")
                nc.tensor.matmul(
                    muT[:], lhsT=wt_t[:], rhs=xts[:, ts(i, 512)],
                    start=True, stop=True,
                )
                nc.vector.tensor_copy(mu2e[0:DSP, :], muT[:])
                if bias_nonzero:
                    sbc = pps.tile([DSP, 512], f32, tag="sbc")
                    nc.tensor.matmul(sbc[:], lhsT=onesr[:], rhs=snd[:],
                                     start=True, stop=True)
                    nc.vector.tensor_tensor(out=mu2e[0:DSP, :],
                                            in0=mu2e[0:DSP, :],
                                            in1=sbc[:], op=OP.mult)
                    nc.vector.tensor_scalar(
                        out=mu2e[0:DSP, :], in0=mu2e[0:DSP, :],
                        scalar1=b_col[:], scalar2=None, op0=OP.add,
                    )
                sq = p1.tile([DSP, 512], f32, tag="sq")
                nc.vector.tensor_tensor(out=sq[:], in0=mu2e[0:DSP, :],
                                        in1=mu2e[0:DSP, :], op=OP.mult)
                msq = pps.tile([1, 512], f32, tag="msq")
                nc.tensor.matmul(msq[:], lhsT=ones64[:], rhs=sq[:],
                                 start=True, stop=True)

                lnm = p1.tile([1, 512], f32, tag="lnm")
                nc.scalar.activation(lnm[:], msq[:], AF.Ln)
                r0 = p1.tile([1, 512], f32, tag="r0")
                nc.scalar.activation(r0[:], lnm[:], AF.Exp, scale=0.5)
                if not bias_nonzero:
                    nc.vector.tensor_tensor(out=r0[:], in0=r0[:],
                                            in1=snd[:], op=OP.mult)
                rc = p1.tile([1, 512], f32, tag="rc")
                nc.vector.tensor_scalar_max(rc[:], r0[:], 1e-10)
                ep = p1.tile([1, 512], f32, tag="ep")
                nc.scalar.activation(ep[:], rc[:], AF.Exp)
                em = p1.tile([1, 512], f32, tag="em")
                nc.scalar.activation(em[:], rc[:], AF.Exp, scale=-1.0)
                y0 = p1.tile([1, 512], f32, tag="y0")
                nc.vector.tensor_tensor(out=y0[:], in0=ep[:], in1=em[:],
                                        op=OP.add)
                nc.vector.tensor_scalar_mul(y0[:], y0[:], 0.5)
                f0 = p1.tile([1, 512], f32, tag="f0")
                nc.vector.tensor_tensor(out=f0[:], in0=ep[:], in1=em[:],
                                        op=OP.subtract)
                rci = p1.tile([1, 512], f32, tag="rci")
                nc.vector.reciprocal(rci[:], rc[:])
                g = p1.tile([1, 512], f32, tag="g")
                nc.vector.tensor_tensor(out=g[:], in0=f0[:], in1=rci[:],
                                        op=OP.mult)
                if not bias_nonzero:
                    nc.vector.tensor_tensor(out=g[:], in0=g[:], in1=snd[:],
                                            op=OP.mult)
                nc.vector.tensor_scalar_mul(sny0[0:1, :], g[:], 0.5)
                nc.sync.dma_start(sny0[1:2, :], y0[:])

                # bc rows 0..63 = sn broadcast, row 64 = y0
                bc = pps.tile([DSP + 1, 512], f32, tag="bc")
                nc.tensor.matmul(bc[:], lhsT=bc2[:], rhs=sny0[:],
                                 start=True, stop=True)
                nc.vector.tensor_tensor(
                    out=tblT[0 : DSP + 1, ts(i, 512)], in0=mu2e[:],
                    in1=bc[:], op=OP.mult,
                )

            nc.sync.dma_start(tbl_in, tblT[0:66, :])

        tc.strict_bb_all_engine_barrier()
        nc.gpsimd.collective_compute(
            "AllGather",
            mybir.AluOpType.bypass,
            replica_groups=[list(range(NCORES))],
            ins=[tbl_in],
            outs=[tbl_all],
        )
        tc.strict_bb_all_engine_barrier()

        # ---------------- Phase 2: gather + Lorentz inner product ----------
        pg = ctx.enter_context(tc.tile_pool(name="pg", bufs=1))
        pq = ctx.enter_context(tc.tile_pool(name="pq", bufs=1, space="PSUM"))

        A = pg.tile([80, E_gt], f32)
        B = pg.tile([80, E_gt], f32)
        Tdst = pg.tile([80, BLK], f32)
        nc.gpsimd.memset(Tdst[64:80, :], 0.0)
        ia = pg.tile([80, GW], i16)
        ib = pg.tile([80, GW], i16)
        rs = pg.tile([1, E_gt], f16)

        def p2_body(nt, tdst_src):
            nc.sync.dma_start(Tdst[0:66, :], tdst_src)
            nc.sync.dma_start(ia[0:16, :], idxs_d[ts(nt, 16), :])
            nc.sync.dma_start(ib[0:16, :], idxd_d[ts(nt, 16), :])
            for t in (ia, ib):
                nc.sync.dma_start(t[16:32, :], t[0:16, :])
                nc.sync.dma_start(t[32:64, :], t[0:32, :])
                nc.sync.dma_start(t[64:80, :], t[0:16, :])
            nc.gpsimd.ap_gather(
                B[:], Tdst[:], ib[:],
                channels=80, num_elems=BLK, d=1, num_idxs=E_gt,
            )
            nc.gpsimd.ap_gather(
                A[:], tblT[:], ia[:],
                channels=80, num_elems=BLK, d=1, num_idxs=E_gt,
            )
            nc.vector.tensor_tensor(
                out=A[0:66, :], in0=A[0:66, :], in1=B[0:66, :], op=OP.mult
            )
            NR = NQ // 8          # full rounds of 8 chunks
            def round_body(roff, rdst, nch):
                ps = pq.tile([1, 4096], f32, tag="ps")
                for k in range(nch):
                    nc.tensor.matmul(
                        ps[:, k * 512 : (k + 1) * 512], lhsT=wvec[:],
                        rhs=roff[:, k * 512 : (k + 1) * 512],
                        start=True, stop=True,
                    )
                w = nch * 512
                nc.vector.tensor_scalar(
                    out=ps[:, 0:w], in0=ps[:, 0:w], scalar1=1e-10,
                    scalar2=1.0, op0=OP.max, op1=OP.min,
                )
                nc.scalar.activation(rdst, ps[:, 0:w], AF.Exp, scale=-1.0)

            if NR:
                with tc.For_i(0, NR) as r:
                    round_body(A[0:66, ts(r, 4096)], rs[:, ts(r, 4096)], 8)
            if NQ % 8:
                round_body(A[0:66, NR * 4096 : NQ * 512],
                           rs[:, NR * 4096 : NQ * 512], NQ % 8)
            nc.sync.dma_start(res[ds(nt, 1), :], rs[:])

        if T == 1:
            for _ in range(reps):
                with tc.For_i(0, NT) as nt:
                    p2_body(nt, tbl_all[ts(nt, 66), :])
        else:
            for j in range(NCORES):
                with tc.For_i(j * T, (j + 1) * T) as nt:
                    p2_body(nt, tbl_all[j * 66 : (j + 1) * 66, :])

    nc.compile()
    return nc


def kernel(x, weight, bias, adj_indices):
    from concourse.bass_utils import run_bass_kernel_spmd

    x = np.asarray(x, dtype=np.float32)
    weight = np.asarray(weight, dtype=np.float32)
    bias_np = np.asarray(bias, dtype=np.float32)
    adj = np.asarray(adj_indices)
    Eall = adj.shape[1]
    src = adj[0].astype(np.int64)
    dst = adj[1].astype(np.int64)

    # ---- host prep: bucket edges by (src block, dst block) ----
    key = ((src // BLK) * NCORES + (dst // BLK)).astype(np.int32)
    order = np.argsort(key, kind="stable")
    counts = np.bincount(key, minlength=NCORES * NCORES).reshape(
        NCORES, NCORES
    )
    starts = np.zeros(NCORES * NCORES + 1, dtype=np.int64)
    np.cumsum(counts.reshape(-1), out=starts[1:])
    E_gmax = int(counts.max())
    T = max(1, -(-E_gmax // CAP))
    per_t = -(-E_gmax // T)
    E_gt = max(-(-per_t // 512) * 512, 512)
    GW = E_gt // 16
    NT = NCORES * T

    # ---- per-core inputs ----
    xp = np.zeros((NPAD, DSP + 1), dtype=np.float32)
    xp[:N] = x
    xp[N:, 0] = 1.0
    wt = np.ascontiguousarray(weight.T)                   # [k, j]
    b_in = np.ascontiguousarray(bias_np.reshape(1, DSP))
    wv_host = np.full((66, 1), -1.0, dtype=np.float32)
    wv_host[64, 0] = 1.0
    bc2_host = np.zeros((2, DSP + 1), dtype=np.float32)
    bc2_host[0, 0:DSP] = 1.0
    bc2_host[1, DSP] = 1.0
    bias_nonzero = bool(np.any(bias_np != 0.0))

    in_maps = []
    sels = []
    for c in range(NCORES):
        idx_s = np.zeros((NT, 16, GW), dtype=np.int16)
        idx_d = np.zeros((NT, 16, GW), dtype=np.int16)
        sel_c = []
        for j in range(NCORES):
            k = c * NCORES + j
            cnt = int(counts[c, j])
            sel = order[starts[k] : starts[k] + cnt]
            s_rel = (src[sel] - c * BLK).astype(np.int16)
            d_rel = (dst[sel] - j * BLK).astype(np.int16)
            for t in range(T):
                lo, hi = t * E_gt, min((t + 1) * E_gt, cnt)
                nt = j * T + t
                if lo >= cnt:
                    sel_c.append((nt, None))
                    continue
                n = hi - lo
                sp = np.zeros(E_gt, dtype=np.int16)
                dp = np.zeros(E_gt, dtype=np.int16)
                sp[:n] = s_rel[lo:hi]
                dp[:n] = d_rel[lo:hi]
                idx_s[nt] = sp.reshape(GW, 16).T
                idx_d[nt] = dp.reshape(GW, 16).T
                sel_c.append((nt, sel[lo:hi]))
        sels.append(sel_c)
        c0 = c * BLK
        in_maps.append({
            "xT": np.ascontiguousarray(xp[c0 : c0 + BLK].T),
            "wt": wt,
            "bias": b_in,
            "idxs": idx_s.reshape(NT * 16, GW),
            "idxd": idx_d.reshape(NT * 16, GW),
            "wv": wv_host,
            "bc2": bc2_host,
        })

    key_p = (E_gt, T, bias_nonzero, REPS)
    if key_p not in _prog_cache:
        _prog_cache[key_p] = _build_program(E_gt, T, bias_nonzero, REPS)
    nc = _prog_cache[key_p]

    import sys

    _self = sys.modules[__name__]  # stash run args/results for the harness
    _self.LAST_ARGS = (nc, in_maps)
    robj = run_bass_kernel_spmd(nc, in_maps, list(range(NCORES)))
    _self.LAST_RUN = robj
    results = robj.results

    # ---- host reassembly ----
    out = np.empty(Eall, dtype=np.float32)
    for c in range(NCORES):
        r = results[c]["res"]  # [NT, E_gt] f16
        for nt, sel in sels[c]:
            if sel is None:
                continue
            out[sel] = r[nt, : len(sel)].astype(np.float32)
    return out


# revision 4
# speedup vs baseline: 1.2651x; 1.0459x over previous
# Trainium2 Bass kernel for nn_LorentzSparseSqDisAtt (GNN edge attention), v3.
#
# Algorithm (8 cores, full I/O):
#   Nodes padded to 53248 and split into 8 blocks of 6656. Edges sharded by
#   src-block (core c owns edges with src in block c), bucketed by dst-block.
#   Phase 1 (sharded, transposed): core c builds the feature-major table
#     for its own block: muT = W @ xt via PE (static weights), per-node
#     scalar chain on [1,512] rows, per-node scales broadcast to 64 rows
#     via K=1 matmuls. Table rows: 0..63 tail, 64 y0, 65 const 1.0,
#     66..79 zero. AllGather -> full table on every core.
#   Phase 2: per dst-block: load block table to SBUF, gpsimd.ap_gather both
#     endpoints (block-relative idx fit int16), DVE product, PE ones-matmul
#     with weights [-1 x64, +1, -1] gives t = y0s*y0d - ts.td - 1 directly,
#     DVE clip, ACT exp -> f16.
#   All repetitive work runs under tc.For_i hardware loops: per-program-
#   instruction dispatch costs ~60us in this runtime, so the program is
#   kept to ~100 instructions.
import numpy as np

N = 50000
DSP = 64          # spatial dim
NCORES = 8
BLK = 6656        # 13 * 512 nodes per block
NPAD = BLK * NCORES
NW = BLK // 512   # phase-1 iterations (512 nodes each)
CAP = 14336       # max edges per gather tile (SBUF bound)

_prog_cache = {}
REPS = 1


def _build_program(E_gt, T, bias_nonzero, reps=1):
    from contextlib import ExitStack

    import concourse.bacc as bacc
    import concourse.bass as bass
    import concourse.tile as tile
    from concourse import mybir
    from concourse.bass import ds, ts

    f32 = mybir.dt.float32
    f16 = mybir.dt.float16
    i16 = mybir.dt.int16
    AF = mybir.ActivationFunctionType
    OP = mybir.AluOpType

    GW = E_gt // 16
    NT = NCORES * T           # gather tiles per core
    NQ = E_gt // 512          # matmul chunks per tile

    nc = bacc.Bacc(
        "TRN2",
        target_bir_lowering=False,
        debug=False,
        enable_asserts=False,
        num_devices=NCORES,
    )

    xti_d = nc.dram_tensor("xti", [DSP, BLK], i16, kind="ExternalInput").ap()
    x0f_d = nc.dram_tensor("x0f", [1, BLK], f32, kind="ExternalInput").ap()
    qcol_d = nc.dram_tensor("qcol", [DSP, 1], f32, kind="ExternalInput").ap()
    wt = nc.dram_tensor("wt", [DSP, DSP], f32, kind="ExternalInput").ap()
    bias_d = nc.dram_tensor("bias", [1, DSP], f32, kind="ExternalInput").ap()
    idxs_d = nc.dram_tensor("idxs", [NT * 16, GW], i16,
                            kind="ExternalInput").ap()
    idxd_d = nc.dram_tensor("idxd", [NT * 16, GW], i16,
                            kind="ExternalInput").ap()
    wv_d = nc.dram_tensor("wv", [66, 1], f32, kind="ExternalInput").ap()
    bc2_d = nc.dram_tensor("bc2", [2, DSP + 1], f32, kind="ExternalInput").ap()
    res = nc.dram_tensor("res", [NT, E_gt], f16, kind="ExternalOutput").ap()
    tbl_in = nc.dram_tensor("tbl_in", [66, BLK], f32).ap()
    tbl_all = nc.dram_tensor(
        "tbl_all", [NCORES * 66, BLK], f32, addr_space="Shared"
    ).ap()

    with tile.TileContext(nc) as tc, ExitStack() as ctx:
        cpool = ctx.enter_context(tc.tile_pool(name="const", bufs=1))

        tblT = cpool.tile([80, BLK], f32)       # my block, feature-major
        nc.gpsimd.memset(tblT[64:80, :], 0.0)
        o_row = cpool.tile([1, BLK], f32)
        nc.gpsimd.memset(o_row[:], 1.0)
        nc.sync.dma_start(tblT[65:66, :], o_row[:])   # table row 65 == 1.0
        wvec = cpool.tile([66, 1], f32)
        nc.sync.dma_start(wvec[:], wv_d)
        neg1 = cpool.tile([1, 1], f32)
        nc.gpsimd.memset(neg1[:], -1.0)
        ones64 = cpool.tile([DSP, 1], f32)
        nc.gpsimd.memset(ones64[:], 1.0)
        onesr = cpool.tile([1, DSP], f32)
        nc.gpsimd.memset(onesr[:], 1.0)
        bc2 = cpool.tile([2, DSP + 1], f32)
        nc.sync.dma_start(bc2[:], bc2_d)
        wt_t = cpool.tile([DSP, DSP], f32)
        nc.sync.dma_start(wt_t[:], wt)

        # ---------------- Phase 1: build my block's table ----------------
        with ExitStack() as p1ctx:
            p1 = p1ctx.enter_context(tc.tile_pool(name="p1", bufs=1))
            pps = p1ctx.enter_context(
                tc.tile_pool(name="pps", bufs=1, space="PSUM")
            )
            p1c = p1ctx.enter_context(tc.tile_pool(name="p1c", bufs=1))

            xti = p1c.tile([DSP, BLK], i16)
            nc.sync.dma_start(xti[:], xti_d)
            qcol = p1c.tile([DSP, 1], f32)
            nc.sync.dma_start(qcol[:], qcol_d)
            xts = p1c.tile([DSP, BLK], f32)
            nc.vector.tensor_scalar(
                out=xts[:], in0=xti[:], scalar1=qcol[:], scalar2=None,
                op0=OP.mult,
            )
            x0r = p1c.tile([1, BLK], f32)
            nc.sync.dma_start(x0r[:], x0f_d)
            if bias_nonzero:
                # b_col[f] = bias[f]: transpose the [1,64] row via PE
                b_row = p1c.tile([1, DSP], f32)
                nc.sync.dma_start(b_row[:], bias_d)
                one1 = p1c.tile([1, 1], f32)
                nc.gpsimd.memset(one1[:], 1.0)
                bT = pps.tile([DSP, 1], f32, tag="bT")
                nc.tensor.matmul(bT[:], lhsT=b_row[:], rhs=one1[:],
                                 start=True, stop=True)
                b_col = p1c.tile([DSP, 1], f32)
                nc.vector.tensor_copy(b_col[:], bT[:])

            mu2e = p1.tile([DSP + 1, 512], f32)
            nc.gpsimd.memset(mu2e[64:65, :], 1.0)
            sny0 = p1.tile([2, 512], f32)

            # Single For_i body; ACT uses only {Ln, Exp} (one table set):
            # sqrt(x) computed as exp(0.5*ln(x)).
            with tc.For_i(0, NW) as i:
                x0c = x0r[:, ts(i, 512)]             # [1, 512]
                z = p1.tile([1, 512], f32, tag="z")
                nc.vector.tensor_scalar_max(z[:], x0c, 1.0 + 1e-7)
                zsq = p1.tile([1, 512], f32, tag="zsq")
                nc.vector.tensor_tensor(out=zsq[:], in0=z[:], in1=z[:],
                                        op=OP.mult)
                lnu = p1.tile([1, 512], f32, tag="lnu")
                nc.scalar.activation(lnu[:], zsq[:], AF.Ln, bias=neg1[:])
                w0 = p1.tile([1, 512], f32, tag="w0")
                nc.scalar.activation(w0[:], lnu[:], AF.Exp, scale=0.5)
                zw = p1.tile([1, 512], f32, tag="zw")
                nc.vector.tensor_tensor(out=zw[:], in0=z[:], in1=w0[:],
                                        op=OP.add)
                dist = p1.tile([1, 512], f32, tag="dist")
                nc.scalar.activation(dist[:], zw[:], AF.Ln)
                wc = p1.tile([1, 512], f32, tag="wc")
                nc.vector.tensor_scalar_max(wc[:], w0[:], 1e-10)
                wci = p1.tile([1, 512], f32, tag="wci")
                nc.vector.reciprocal(wci[:], wc[:])
                snd = p1.tile([1, 512], f32, tag="snd")
                nc.vector.tensor_tensor(out=snd[:], in0=dist[:], in1=wci[:],
                                        op=OP.mult)

                muT = pps.tile([DSP, 512], f32, tag="muT")
                nc.tensor.matmul(
                    muT[:], lhsT=wt_t[:], rhs=xts[:, ts(i, 512)],
                    start=True, stop=True,
                )
                nc.vector.tensor_copy(mu2e[0:DSP, :], muT[:])
                if bias_nonzero:
                    sbc = pps.tile([DSP, 512], f32, tag="sbc")
                    nc.tensor.matmul(sbc[:], lhsT=onesr[:], rhs=snd[:],
                                     start=True, stop=True)
                    nc.vector.tensor_tensor(out=mu2e[0:DSP, :],
                                            in0=mu2e[0:DSP, :],
                                            in1=sbc[:], op=OP.mult)
                    nc.vector.tensor_scalar(
                        out=mu2e[0:DSP, :], in0=mu2e[0:DSP, :],
                        scalar1=b_col[:], scalar2=None, op0=OP.add,
                    )
                sq = p1.tile([DSP, 512], f32, tag="sq")
                nc.vector.tensor_tensor(out=sq[:], in0=mu2e[0:DSP, :],
                                        in1=mu2e[0:DSP, :], op=OP.mult)
                msq = pps.tile([1, 512], f32, tag="msq")
                nc.tensor.matmul(msq[:], lhsT=ones64[:], rhs=sq[:],
                                 start=True, stop=True)

                lnm = p1.tile([1, 512], f32, tag="lnm")
                nc.scalar.activation(lnm[:], msq[:], AF.Ln)
                r0 = p1.tile([1, 512], f32, tag="r0")
                nc.scalar.activation(r0[:], lnm[:], AF.Exp, scale=0.5)
                if not bias_nonzero:
                    nc.vector.tensor_tensor(out=r0[:], in0=r0[:],
                                            in1=snd[:], op=OP.mult)
                rc = p1.tile([1, 512], f32, tag="rc")
                nc.vector.tensor_scalar_max(rc[:], r0[:], 1e-10)
                ep = p1.tile([1, 512], f32, tag="ep")
                nc.scalar.activation(ep[:], rc[:], AF.Exp)
                em = p1.tile([1, 512], f32, tag="em")
                nc.scalar.activation(em[:], rc[:], AF.Exp, scale=-1.0)
                y0 = p1.tile([1, 512], f32, tag="y0")
                nc.vector.tensor_tensor(out=y0[:], in0=ep[:], in1=em[:],
                                        op=OP.add)
                nc.vector.tensor_scalar_mul(y0[:], y0[:], 0.5)
                f0 = p1.tile([1, 512], f32, tag="f0")
                nc.vector.tensor_tensor(out=f0[:], in0=ep[:], in1=em[:],
                                        op=OP.subtract)
                rci = p1.tile([1, 512], f32, tag="rci")
                nc.vector.reciprocal(rci[:], rc[:])
                g = p1.tile([1, 512], f32, tag="g")
                nc.vector.tensor_tensor(out=g[:], in0=f0[:], in1=rci[:],
                                        op=OP.mult)
                if not bias_nonzero:
                    nc.vector.tensor_tensor(out=g[:], in0=g[:], in1=snd[:],
                                            op=OP.mult)
                nc.vector.tensor_scalar_mul(sny0[0:1, :], g[:], 0.5)
                nc.sync.dma_start(sny0[1:2, :], y0[:])

                # bc rows 0..63 = sn broadcast, row 64 = y0
                bc = pps.tile([DSP + 1, 512], f32, tag="bc")
                nc.tensor.matmul(bc[:], lhsT=bc2[:], rhs=sny0[:],
                                 start=True, stop=True)
                nc.vector.tensor_tensor(
                    out=tblT[0 : DSP + 1, ts(i, 512)], in0=mu2e[:],
                    in1=bc[:], op=OP.mult,
                )

            nc.sync.dma_start(tbl_in, tblT[0:66, :])

        tc.strict_bb_all_engine_barrier()
        nc.gpsimd.collective_compute(
            "AllGather",
            mybir.AluOpType.bypass,
            replica_groups=[list(range(NCORES))],
            ins=[tbl_in],
            outs=[tbl_all],
        )
        tc.strict_bb_all_engine_barrier()

        # ---------------- Phase 2: gather + Lorentz inner product ----------
        pg = ctx.enter_context(tc.tile_pool(name="pg", bufs=1))
        pq = ctx.enter_context(tc.tile_pool(name="pq", bufs=1, space="PSUM"))

        A = pg.tile([80, E_gt], f32)
        B = pg.tile([80, E_gt], f32)
        Tdst = pg.tile([80, BLK], f32)
        nc.gpsimd.memset(Tdst[64:80, :], 0.0)
        ia = pg.tile([80, GW], i16)
        ib = pg.tile([80, GW], i16)
        rs = pg.tile([1, E_gt], f16)

        def p2_body(nt, tdst_src):
            nc.sync.dma_start(Tdst[0:66, :], tdst_src)
            nc.sync.dma_start(ia[0:16, :], idxs_d[ts(nt, 16), :])
            nc.sync.dma_start(ib[0:16, :], idxd_d[ts(nt, 16), :])
            for t in (ia, ib):
                nc.sync.dma_start(t[16:32, :], t[0:16, :])
                nc.sync.dma_start(t[32:64, :], t[0:32, :])
                nc.sync.dma_start(t[64:80, :], t[0:16, :])
            nc.gpsimd.ap_gather(
                B[:], Tdst[:], ib[:],
                channels=80, num_elems=BLK, d=1, num_idxs=E_gt,
            )
            nc.gpsimd.ap_gather(
                A[:], tblT[:], ia[:],
                channels=80, num_elems=BLK, d=1, num_idxs=E_gt,
            )
            nc.vector.tensor_tensor(
                out=A[0:66, :], in0=A[0:66, :], in1=B[0:66, :], op=OP.mult
            )
            NR = NQ // 8          # full rounds of 8 chunks
            def round_body(roff, rdst, nch):
                ps = pq.tile([1, 4096], f32, tag="ps")
                for k in range(nch):
                    nc.tensor.matmul(
                        ps[:, k * 512 : (k + 1) * 512], lhsT=wvec[:],
                        rhs=roff[:, k * 512 : (k + 1) * 512],
                        start=True, stop=True,
                    )
                w = nch * 512
                nc.vector.tensor_scalar(
                    out=ps[:, 0:w], in0=ps[:, 0:w], scalar1=1e-10,
                    scalar2=1.0, op0=OP.max, op1=OP.min,
                )
                nc.scalar.activation(rdst, ps[:, 0:w], AF.Exp, scale=-1.0)

            if NR:
                with tc.For_i(0, NR) as r:
                    round_body(A[0:66, ts(r, 4096)], rs[:, ts(r, 4096)], 8)
            if NQ % 8:
                round_body(A[0:66, NR * 4096 : NQ * 512],
                           rs[:, NR * 4096 : NQ * 512], NQ % 8)
            nc.sync.dma_start(res[ds(nt, 1), :], rs[:])

        if T == 1:
            for _ in range(reps):
                with tc.For_i(0, NT) as nt:
                    p2_body(nt, tbl_all[ts(nt, 66), :])
        else:
            for j in range(NCORES):
                with tc.For_i(j * T, (j + 1) * T) as nt:
                    p2_body(nt, tbl_all[j * 66 : (j + 1) * 66, :])

    nc.compile()
    return nc


def kernel(x, weight, bias, adj_indices):
    from concourse.bass_utils import run_bass_kernel_spmd

    x = np.asarray(x, dtype=np.float32)
    weight = np.asarray(weight, dtype=np.float32)
    bias_np = np.asarray(bias, dtype=np.float32)
    adj = np.asarray(adj_indices)
    Eall = adj.shape[1]
    src = adj[0].astype(np.int64)
    dst = adj[1].astype(np.int64)

    # ---- host prep: bucket edges by (src block, dst block) ----
    key = ((src // BLK) * NCORES + (dst // BLK)).astype(np.int32)
    order = np.argsort(key, kind="stable")
    counts = np.bincount(key, minlength=NCORES * NCORES).reshape(
        NCORES, NCORES
    )
    starts = np.zeros(NCORES * NCORES + 1, dtype=np.int64)
    np.cumsum(counts.reshape(-1), out=starts[1:])
    E_gmax = int(counts.max())
    T = max(1, -(-E_gmax // CAP))
    per_t = -(-E_gmax // T)
    E_gt = max(-(-per_t // 512) * 512, 512)
    GW = E_gt // 16
    NT = NCORES * T

    # ---- per-core inputs ----
    xp = np.zeros((NPAD, DSP + 1), dtype=np.float32)
    xp[:N] = x
    xp[N:, 0] = 1.0
    qs = float(np.abs(xp[:, 1:]).max()) / 32000.0
    qs = max(qs, 1e-30)
    xq = np.round(xp[:, 1:] * (1.0 / qs)).astype(np.int16)   # [NPAD, 64]
    qcol_host = np.full((DSP, 1), qs, dtype=np.float32)
    wt = np.ascontiguousarray(weight.T)                   # [k, j]
    b_in = np.ascontiguousarray(bias_np.reshape(1, DSP))
    wv_host = np.full((66, 1), -1.0, dtype=np.float32)
    wv_host[64, 0] = 1.0
    bc2_host = np.zeros((2, DSP + 1), dtype=np.float32)
    bc2_host[0, 0:DSP] = 1.0
    bc2_host[1, DSP] = 1.0
    bias_nonzero = bool(np.any(bias_np != 0.0))

    in_maps = []
    sels = []
    for c in range(NCORES):
        idx_s = np.zeros((NT, 16, GW), dtype=np.int16)
        idx_d = np.zeros((NT, 16, GW), dtype=np.int16)
        sel_c = []
        for j in range(NCORES):
            k = c * NCORES + j
            cnt = int(counts[c, j])
            sel = order[starts[k] : starts[k] + cnt]
            s_rel = (src[sel] - c * BLK).astype(np.int16)
            d_rel = (dst[sel] - j * BLK).astype(np.int16)
            for t in range(T):
                lo, hi = t * E_gt, min((t + 1) * E_gt, cnt)
                nt = j * T + t
                if lo >= cnt:
                    sel_c.append((nt, None))
                    continue
                n = hi - lo
                sp = np.zeros(E_gt, dtype=np.int16)
                dp = np.zeros(E_gt, dtype=np.int16)
                sp[:n] = s_rel[lo:hi]
                dp[:n] = d_rel[lo:hi]
                idx_s[nt] = sp.reshape(GW, 16).T
                idx_d[nt] = dp.reshape(GW, 16).T
                sel_c.append((nt, sel[lo:hi]))
        sels.append(sel_c)
        c0 = c * BLK
        in_maps.append({
            "xti": np.ascontiguousarray(xq[c0 : c0 + BLK].T),
            "x0f": np.ascontiguousarray(xp[c0 : c0 + BLK, 0].reshape(1, BLK)),
            "qcol": qcol_host,
            "wt": wt,
            "bias": b_in,
            "idxs": idx_s.reshape(NT * 16, GW),
            "idxd": idx_d.reshape(NT * 16, GW),
            "wv": wv_host,
            "bc2": bc2_host,
        })

    key_p = (E_gt, T, bias_nonzero, REPS)
    if key_p not in _prog_cache:
        _prog_cache[key_p] = _build_program(E_gt, T, bias_nonzero, REPS)
    nc = _prog_cache[key_p]

    import sys

    _self = sys.modules[__name__]  # stash run args/results for the harness
    _self.LAST_ARGS = (nc, in_maps)
    robj = run_bass_kernel_spmd(nc, in_maps, list(range(NCORES)))
    _self.LAST_RUN = robj
    results = robj.results

    # ---- host reassembly ----
    out = np.empty(Eall, dtype=np.float32)
    for c in range(NCORES):
        r = results[c]["res"]  # [NT, E_gt] f16
        for nt, sel in sels[c]:
            if sel is None:
                continue
            out[sel] = r[nt, : len(sel)].astype(np.float32)
    return out
